# revision 29
# baseline (speedup 1.0000x reference)
"""Distributed attention kernel for Trainium2 (8 NeuronCores, SPMD).

Problem: B=4, S=4096, D=256 attention with QKV linear projections.
Sharding: core = 2*b + half -> batch b, query rows [half*2048, (half+1)*2048),
full K/V for that batch on every core (data-parallel batch + seq-parallel q).

Layout/precision strategy:
- All sequence tensors are pre-transposed on the host to [D, seq] (bf16) so
  every matmul contraction lands on the partition axis; no device transposes.
- Scores are computed transposed (scoresT[j, i] = kp . qp) so softmax exp is
  purely elementwise (ScalarE, reading PSUM, softmax scale folded into Wq)
  and the AV matmul consumes exp tiles directly as the stationary operand.
- qp/kp are quantized to fp8e4 in a K-group-interleaved layout [128, 2, cols]
  so each 256-deep scores contraction is ONE DoubleRow matmul. vp and the
  AV matmul stay bf16 (fp8 there costs too much accuracy).
- vp carries an extra all-ones column so the AV accumulation also produces
  softmax row-sums; normalization is reciprocal+scale on [128, 257] tiles.
  The v-bias (+ ones column) is added by the DVE during the PSUM->SBUF copy
  against a precomputed ones x [bv, 1] rank-1 tile.
- Pipelining: one packed weight-blob DMA; inputs stream in 1024-column chunks
  issued earliest-columns-first; a short TensorE warm-up keeps the HAM clock
  gate open until real data lands; the vp projection is fused into the first
  attention i-chunk so no engine epilogue drains serially.
"""

import math
import time
from contextlib import ExitStack

import numpy as np
import ml_dtypes

B = 4
S = 4096
D = 256
SQ = 2048  # query rows per core
NE = 2  # number of 128-row tiles covering D
ICH = 512  # i-chunk (query columns per scoresT matmul)
NIC = SQ // ICH  # 4
NJ = S // 128  # 32 j-tiles
H1 = D + 1  # vp width including the ones column
BLOBW = 4 * D + 2 * H1  # packed weights blob columns (bf16)
LCH = 1024  # DMA load chunk (columns)
PCH = 512  # projection chunk (columns)

_CACHE = {}
_last_in_maps = None


def _build():
    import concourse.bass as bass
    import concourse.tile as tile
    from concourse import bacc, mybir

    BF = mybir.dt.bfloat16
    F8 = mybir.dt.float8e4
    F32 = mybir.dt.float32
    AF = mybir.ActivationFunctionType

    nc = bacc.Bacc("TRN2", target_bir_lowering=False, debug=False)

    qT = nc.declare_dram_parameter("qT", [D, SQ], BF, isOutput=False)
    kT = nc.declare_dram_parameter("kT", [D, S], BF, isOutput=False)
    vT = nc.declare_dram_parameter("vT", [D, S], BF, isOutput=False)
    blob = nc.declare_dram_parameter("blob", [128, BLOBW], BF, isOutput=False)
    bblob = nc.declare_dram_parameter("bblob", [128, 4], F32, isOutput=False)
    bv1 = nc.declare_dram_parameter("bv1", [1, H1], BF, isOutput=False)
    out = nc.declare_dram_parameter("out", [SQ, D], F32, isOutput=True)

    with TileKernel(nc, tile) as (tc, ctx):
        const = ctx.enter_context(tc.tile_pool(name="const", bufs=1))
        inp = ctx.enter_context(tc.tile_pool(name="inp", bufs=1))
        persist = ctx.enter_context(tc.tile_pool(name="persist", bufs=1))

        # --- PE warm-up: dummy matmuls so HAM un-throttles before data lands ---
        warm = const.tile([128, 512], BF, tag="warm")
        nc.vector.memset(warm[:], 0.0)
        with tc.tile_pool(name="wpsum", bufs=1, space=bass.MemorySpace.PSUM) as wpsum:
            wp = wpsum.tile([128, 512], F32, tag="wp")
            for _ in range(12):
                nc.tensor.matmul(wp[:], warm[:, 0:128], warm[:], start=True,
                                 stop=True, skip_group_check=True)

        # --- all weights/biases in one packed blob: a single DMA issue slot ---
        # layout (bf16 cols): wq[512] wk[512] wv+zeros[514] bqs[2] bk[2]
        blob_sb = const.tile([128, BLOBW], BF, tag="blob")
        nc.gpsimd.dma_start(blob_sb[:], blob[:])
        bblob_sb = const.tile([128, 4], F32, tag="bblob")
        nc.gpsimd.dma_start(bblob_sb[:], bblob[:])
        wq_sb = [blob_sb[:, dt * D:(dt + 1) * D] for dt in range(NE)]
        wk_sb = [blob_sb[:, 2 * D + dt * D:2 * D + (dt + 1) * D] for dt in range(NE)]
        wv_sb = [blob_sb[:, 4 * D + dt * H1:4 * D + (dt + 1) * H1] for dt in range(NE)]
        bqs_sb = [bblob_sb[:, et:et + 1] for et in range(NE)]
        bk_sb = [bblob_sb[:, 2 + et:3 + et] for et in range(NE)]
        bv1_sb = const.tile([1, H1], BF, tag="bv1")
        nc.gpsimd.dma_start(bv1_sb[:], bv1[:])
        ones1 = const.tile([1, 128], BF, tag="ones1")
        nc.vector.memset(ones1[:], 1.0)
        biasones = const.tile([128, H1], F32, tag="biasones")
        with tc.tile_pool(name="bopsum", bufs=1, space=bass.MemorySpace.PSUM) as bop:
            bp = bop.tile([128, H1], F32, tag="bp")
            nc.tensor.matmul(bp[:], ones1[:], bv1_sb[:], start=True, stop=True,
                             skip_group_check=True)
            nc.vector.tensor_copy(biasones[:], bp[:])

        # --- chunked input loads (sync queue = HWDGE) ---
        # per (et, chunk) tiles of [128, LCH]
        def cmake(dram, cols, tag):
            ts = [[None] * (cols // LCH) for _ in range(NE)]
            for c in range(cols // LCH):
                for dt in range(NE):
                    t = inp.tile([128, LCH], BF, tag=f"{tag}{dt}_{c}",
                                 name=f"{tag}{dt}_{c}")
                    ts[dt][c] = t
            return ts

        qT_sb = cmake(qT, SQ, "qT")
        kT_sb = cmake(kT, S, "kT")
        vT_sb = cmake(vT, S, "vT")
        # issue loads chunk-index-first across q/k/v so the columns attention
        # needs first land first
        load_seq = []
        for c in range(S // LCH):
            for dram, ts in ((qT, qT_sb), (kT, kT_sb), (vT, vT_sb)):
                if c < len(ts[0]):
                    load_seq.append((dram, ts, c))
        for dram, ts, c in load_seq:
            for dt in range(NE):
                nc.sync.dma_start(
                    ts[dt][c][:],
                    dram[dt * 128:(dt + 1) * 128, c * LCH:(c + 1) * LCH])

        def in_ap(ts, col0, width):
            """AP into the chunked tiles for [col0, col0+width) per et."""
            c, off = divmod(col0, LCH)
            assert off + width <= LCH
            return [ts[dt][c][:, off:off + width] for dt in range(NE)]

        # --- projections (chunked tiles, emitted in attention-consumption order)
        # qpT/kpT are stored as fp8e4 in K-group-interleaved layout [128, 2, cols]
        # so the scores matmul runs one DoubleRow matmul with K=256.
        qpT_sb = [persist.tile([128, NE, PCH], F8, tag=f"qpT{c}", name=f"qpT{c}")
                  for c in range(SQ // PCH)]
        kpT_sb = [persist.tile([128, NE, PCH], F8, tag=f"kpT{c}", name=f"kpT{c}")
                  for c in range(S // PCH)]
        vp_sb = [persist.tile([128, H1], BF, tag=f"vp{j}", name=f"vp{j}")
                 for j in range(NJ)]

        with tc.tile_pool(name="ppsum", bufs=4, space=bass.MemorySpace.PSUM) as ppsum:
            # qpT[e, i] = sum_d WqT_s[d, e] * qT[d, i]  (+ s*bq)
            def qp_proj(c, psum_pool):
                src = in_ap(qT_sb, c * PCH, PCH)
                for et in range(NE):
                    esl = slice(et * 128, (et + 1) * 128)
                    pp = psum_pool.tile([128, PCH], F32, tag="pv",
                                        name=f"qpp{c}_{et}")
                    nc.tensor.matmul(pp[:], wq_sb[0][:, esl], src[0],
                                     start=True, stop=False)
                    nc.tensor.matmul(pp[:], wq_sb[1][:, esl], src[1],
                                     start=False, stop=True)
                    if (c + et) % 2 == 0:
                        nc.vector.tensor_scalar_add(qpT_sb[c][:, et, :], pp[:],
                                                    bqs_sb[et])
                    else:
                        nc.scalar.activation(qpT_sb[c][:, et, :], pp[:],
                                             AF.Identity, bias=bqs_sb[et])

            for c in range(SQ // PCH):
                qp_proj(c, ppsum)
            # kpT[e, j] = sum_d WkT[d, e] * kT[d, j]  (+ bk)
            for c in range(S // PCH):
                src = in_ap(kT_sb, c * PCH, PCH)
                for et in range(NE):
                    esl = slice(et * 128, (et + 1) * 128)
                    pp = ppsum.tile([128, PCH], F32, tag="pp")
                    nc.tensor.matmul(pp[:], wk_sb[0][:, esl], src[0],
                                     start=True, stop=False)
                    nc.tensor.matmul(pp[:], wk_sb[1][:, esl], src[1],
                                     start=False, stop=True)
                    if (c + et) % 2 == 0:
                        nc.vector.tensor_scalar_add(kpT_sb[c][:, et, :], pp[:],
                                                    bk_sb[et])
                    else:
                        nc.scalar.activation(kpT_sb[c][:, et, :], pp[:],
                                             AF.Identity, bias=bk_sb[et])

        def kp_ap(j):
            """[128, 2, 128] DoubleRow lhsT window of kpT for j-tile j."""
            c, off = divmod(j * 128, PCH)
            return kpT_sb[c][:, :, off:off + 128]

        # --- attention ---
        def attn_chunk(ic, spsum, expp, norm, obuf, opsum, fuse_vp=None):
            qp_rhs = qpT_sb[ic]  # PCH == ICH
            otiles = [opsum.tile([128, H1], F32, tag=f"ot{it}",
                                 name=f"ot{ic}_{it}") for it in range(4)]
            for j in range(NJ):
                if fuse_vp is not None:
                    fuse_vp(j)
                sp = spsum.tile([128, ICH], F32, tag="sp")
                nc.tensor.matmul(sp[:], kp_ap(j), qp_rhs[:],
                                 start=True, stop=True,
                                 perf_mode=mybir.MatmulPerfMode.DoubleRow)
                ex = expp.tile([128, ICH], BF, tag="ex")
                nc.scalar.activation(ex[:], sp[:], AF.Exp)
                for it in range(4):
                    nc.tensor.matmul(otiles[it][:],
                                     ex[:, it * 128:(it + 1) * 128],
                                     vp_sb[j][:],
                                     start=(j == 0), stop=(j == NJ - 1),
                                     skip_group_check=True)
            for it in range(4):
                rt = norm.tile([128, 1], F32, tag="rt")
                nc.vector.reciprocal(rt[:], otiles[it][:, D:H1])
                ob = obuf.tile([128, D], F32, tag="ob")
                nc.vector.tensor_scalar_mul(ob[:], otiles[it][:, 0:D], rt[:])
                r0 = (ic * 4 + it) * 128
                nc.sync.dma_start(out[r0:r0 + 128, :], ob[:])

        with tc.tile_pool(name="opsum", bufs=1, space=bass.MemorySpace.PSUM) as opsum, \
             tc.tile_pool(name="expp", bufs=4) as expp, \
             tc.tile_pool(name="norm", bufs=4) as norm, \
             tc.tile_pool(name="obuf", bufs=4) as obuf:
            # ic0: vp projection fused per-j (copies on DVE) so nothing drains
            # serially before attention starts
            with tc.tile_pool(name="spsum0", bufs=2,
                              space=bass.MemorySpace.PSUM) as spsum0, \
                 tc.tile_pool(name="vpsum", bufs=2,
                              space=bass.MemorySpace.PSUM) as vpsum:
                def vp_proj(j):
                    src2 = in_ap(vT_sb, j * 128, 128)
                    pv = vpsum.tile([128, H1], F32, tag="pv", name=f"pv{j}")
                    nc.tensor.matmul(pv[:], src2[0], wv_sb[0],
                                     start=True, stop=False, skip_group_check=True)
                    nc.tensor.matmul(pv[:], src2[1], wv_sb[1],
                                     start=False, stop=True, skip_group_check=True)
                    nc.vector.tensor_tensor(vp_sb[j][:], pv[:], biasones[:],
                                            mybir.AluOpType.add)

                # prefetch distance 2 so the vp copy never gates the AV of
                # the same iteration
                vp_proj(0)
                vp_proj(1)

                def fuse_vp(j):
                    if j + 2 < NJ:
                        vp_proj(j + 2)

                attn_chunk(0, spsum0, expp, norm, obuf, opsum, fuse_vp=fuse_vp)
            with tc.tile_pool(name="spsum1", bufs=4,
                              space=bass.MemorySpace.PSUM) as spsum1:
                for ic in range(1, NIC):
                    attn_chunk(ic, spsum1, expp, norm, obuf, opsum)

    nc.compile()
    return nc


class TileKernel:
    """Helper: TileContext + ExitStack as one context manager."""

    def __init__(self, nc, tile_mod):
        self.nc = nc
        self.tile_mod = tile_mod
        self.stack = ExitStack()

    def __enter__(self):
        tc = self.stack.enter_context(self.tile_mod.TileContext(self.nc))
        return tc, self.stack

    def __exit__(self, *exc):
        return self.stack.__exit__(*exc)


def _get_nc():
    if "nc" not in _CACHE:
        _CACHE["nc"] = _build()
    return _CACHE["nc"]


def kernel(q, k, v, Wq, bq, Wk, bk, Wv, bv):
    from concourse.bass_utils import run_bass_kernel_spmd

    q = np.asarray(q, dtype=np.float32)
    k = np.asarray(k, dtype=np.float32)
    v = np.asarray(v, dtype=np.float32)
    Wq = np.asarray(Wq, dtype=np.float32)
    bq = np.asarray(bq, dtype=np.float32)
    Wk = np.asarray(Wk, dtype=np.float32)
    bk = np.asarray(bk, dtype=np.float32)
    Wv = np.asarray(Wv, dtype=np.float32)
    bv = np.asarray(bv, dtype=np.float32)

    bf = ml_dtypes.bfloat16
    s = 1.0 / math.sqrt(D)

    WqT = (s * Wq.T).astype(bf)          # [d, e], softmax scale folded in
    WkT = Wk.T.astype(bf)                # [d, e]
    WvT = Wv.T.astype(bf)                # [d, h]
    bv1 = np.concatenate([bv, np.ones(1, np.float32)]).reshape(1, H1).astype(bf)

    blob = np.zeros((128, BLOBW), bf)
    for dt in range(NE):
        blob[:, dt * D:(dt + 1) * D] = WqT[dt * 128:(dt + 1) * 128, :]
        blob[:, 2 * D + dt * D:2 * D + (dt + 1) * D] = WkT[dt * 128:(dt + 1) * 128, :]
        blob[:, 4 * D + dt * H1:4 * D + dt * H1 + D] = WvT[dt * 128:(dt + 1) * 128, :]
    bblob = np.zeros((128, 4), np.float32)
    bqs_full = (s * bq).astype(np.float32)
    for et in range(NE):
        bblob[:, et] = bqs_full[et * 128:(et + 1) * 128]
        bblob[:, 2 + et] = bk[et * 128:(et + 1) * 128]

    shared = dict(blob=blob, bblob=bblob, bv1=bv1)
    in_maps = []
    for core in range(8):
        b, half = divmod(core, 2)
        qs = slice(half * SQ, (half + 1) * SQ)
        in_maps.append(dict(
            qT=q[b, qs, :].T.astype(bf),
            kT=k[b].T.astype(bf),
            vT=v[b].T.astype(bf),
            **shared,
        ))

    global _last_in_maps
    _last_in_maps = in_maps

    nc = _get_nc()
    res = None
    for attempt in range(3):
        try:
            res = run_bass_kernel_spmd(nc, in_maps, core_ids=list(range(8)))
            break
        except Exception:
            if attempt == 2:
                raise
            time.sleep(75)  # axon terminal occasionally wedges; it self-heals

    full = np.empty((B, S, D), np.float32)
    for core in range(8):
        b, half = divmod(core, 2)
        full[b, half * SQ:(half + 1) * SQ, :] = res.results[core]["out"]
    return full


# revision 30
# speedup vs baseline: 1.0481x; 1.0481x over previous
"""Distributed attention kernel for Trainium2 (8 NeuronCores, SPMD).

Problem: B=4, S=4096, D=256 attention with QKV linear projections.
Sharding: core = 2*b + half -> batch b, query rows [half*2048, (half+1)*2048),
full K/V for that batch on every core (data-parallel batch + seq-parallel q).

Layout/precision strategy:
- All sequence tensors are pre-transposed on the host to [D, seq] (bf16) so
  every matmul contraction lands on the partition axis; no device transposes.
- Scores are computed transposed (scoresT[j, i] = kp . qp) so softmax exp is
  purely elementwise (ScalarE, reading PSUM, softmax scale folded into Wq)
  and the AV matmul consumes exp tiles directly as the stationary operand.
- qp/kp are quantized to fp8e4 in a K-group-interleaved layout [128, 2, cols]
  so each 256-deep scores contraction is ONE DoubleRow matmul. vp and the
  AV matmul stay bf16 (fp8 there costs too much accuracy).
- vp carries an extra all-ones column so the AV accumulation also produces
  softmax row-sums; normalization is reciprocal+scale on [128, 257] tiles.
  The v-bias (+ ones column) is added by the DVE during the PSUM->SBUF copy
  against a precomputed ones x [bv, 1] rank-1 tile.
- Pipelining: one packed weight-blob DMA; inputs stream in 1024-column chunks
  issued earliest-columns-first; a short TensorE warm-up keeps the HAM clock
  gate open until real data lands; the vp projection is fused into the first
  attention i-chunk so no engine epilogue drains serially.
"""

import math
import time
from contextlib import ExitStack

import numpy as np
import ml_dtypes

B = 4
S = 4096
D = 256
SQ = 2048  # query rows per core
NE = 2  # number of 128-row tiles covering D
ICH = 512  # i-chunk (query columns per scoresT matmul)
NIC = SQ // ICH  # 4
NJ = S // 128  # 32 j-tiles
H1 = D + 1  # vp width including the ones column
BLOBW = 4 * D + 2 * H1  # packed weights blob columns (bf16)
LCH = 1024  # DMA load chunk (columns)
PCH = 512  # projection chunk (columns)

_CACHE = {}
_last_in_maps = None


def _build():
    import concourse.bass as bass
    import concourse.tile as tile
    from concourse import bacc, mybir

    BF = mybir.dt.bfloat16
    F8 = mybir.dt.float8e4
    F32 = mybir.dt.float32
    AF = mybir.ActivationFunctionType

    nc = bacc.Bacc("TRN2", target_bir_lowering=False, debug=False)

    qT = nc.declare_dram_parameter("qT", [D, SQ], BF, isOutput=False)
    kT = nc.declare_dram_parameter("kT", [D, S], BF, isOutput=False)
    vT = nc.declare_dram_parameter("vT", [D, S], BF, isOutput=False)
    blob = nc.declare_dram_parameter("blob", [128, BLOBW], BF, isOutput=False)
    bblob = nc.declare_dram_parameter("bblob", [128, 4], F32, isOutput=False)
    bv1 = nc.declare_dram_parameter("bv1", [1, H1], BF, isOutput=False)
    out = nc.declare_dram_parameter("out", [SQ, D], F32, isOutput=True)

    with TileKernel(nc, tile) as (tc, ctx):
        const = ctx.enter_context(tc.tile_pool(name="const", bufs=1))
        inp = ctx.enter_context(tc.tile_pool(name="inp", bufs=1))
        persist = ctx.enter_context(tc.tile_pool(name="persist", bufs=1))

        # --- PE warm-up: dummy matmuls so HAM un-throttles before data lands ---
        warm = const.tile([128, 512], BF, tag="warm")
        nc.vector.memset(warm[:], 0.0)
        with tc.tile_pool(name="wpsum", bufs=1, space=bass.MemorySpace.PSUM) as wpsum:
            wp = wpsum.tile([128, 512], F32, tag="wp")
            for _ in range(12):
                nc.tensor.matmul(wp[:], warm[:, 0:128], warm[:], start=True,
                                 stop=True, skip_group_check=True)

        # --- all weights/biases in one packed blob: a single DMA issue slot ---
        # layout (bf16 cols): wq[512] wk[512] wv+zeros[514] bqs[2] bk[2]
        blob_sb = const.tile([128, BLOBW], BF, tag="blob")
        nc.gpsimd.dma_start(blob_sb[:], blob[:])
        bblob_sb = const.tile([128, 4], F32, tag="bblob")
        nc.gpsimd.dma_start(bblob_sb[:], bblob[:])
        wq_sb = [blob_sb[:, dt * D:(dt + 1) * D] for dt in range(NE)]
        wk_sb = [blob_sb[:, 2 * D + dt * D:2 * D + (dt + 1) * D] for dt in range(NE)]
        wv_sb = [blob_sb[:, 4 * D + dt * H1:4 * D + (dt + 1) * H1] for dt in range(NE)]
        bqs_sb = [bblob_sb[:, et:et + 1] for et in range(NE)]
        bk_sb = [bblob_sb[:, 2 + et:3 + et] for et in range(NE)]
        bv1_sb = const.tile([1, H1], BF, tag="bv1")
        nc.gpsimd.dma_start(bv1_sb[:], bv1[:])
        ones1 = const.tile([1, 128], BF, tag="ones1")
        nc.vector.memset(ones1[:], 1.0)
        biasones = const.tile([128, H1], F32, tag="biasones")
        with tc.tile_pool(name="bopsum", bufs=1, space=bass.MemorySpace.PSUM) as bop:
            bp = bop.tile([128, H1], F32, tag="bp")
            nc.tensor.matmul(bp[:], ones1[:], bv1_sb[:], start=True, stop=True,
                             skip_group_check=True)
            nc.vector.tensor_copy(biasones[:], bp[:])

        # --- chunked input loads (sync queue = HWDGE) ---
        # per (et, chunk) tiles of [128, LCH]
        def cmake(dram, cols, tag):
            ts = [[None] * (cols // LCH) for _ in range(NE)]
            for c in range(cols // LCH):
                for dt in range(NE):
                    t = inp.tile([128, LCH], BF, tag=f"{tag}{dt}_{c}",
                                 name=f"{tag}{dt}_{c}")
                    ts[dt][c] = t
            return ts

        qT_sb = cmake(qT, SQ, "qT")
        kT_sb = cmake(kT, S, "kT")
        vT_sb = cmake(vT, S, "vT")
        # issue loads chunk-index-first across q/k/v so the columns attention
        # needs first land first
        load_seq = []
        for c in range(S // LCH):
            for dram, ts in ((qT, qT_sb), (kT, kT_sb), (vT, vT_sb)):
                if c < len(ts[0]):
                    load_seq.append((dram, ts, c))
        for dram, ts, c in load_seq:
            for dt in range(NE):
                nc.sync.dma_start(
                    ts[dt][c][:],
                    dram[dt * 128:(dt + 1) * 128, c * LCH:(c + 1) * LCH])

        def in_ap(ts, col0, width):
            """AP into the chunked tiles for [col0, col0+width) per et."""
            c, off = divmod(col0, LCH)
            assert off + width <= LCH
            return [ts[dt][c][:, off:off + width] for dt in range(NE)]

        # --- projections (chunked tiles, emitted in attention-consumption order)
        # qpT/kpT are stored as fp8e4 in K-group-interleaved layout [128, 2, cols]
        # so the scores matmul runs one DoubleRow matmul with K=256.
        qpT_sb = [persist.tile([128, NE, PCH], F8, tag=f"qpT{c}", name=f"qpT{c}")
                  for c in range(SQ // PCH)]
        kpT_sb = [persist.tile([128, NE, PCH], F8, tag=f"kpT{c}", name=f"kpT{c}")
                  for c in range(S // PCH)]
        vp_sb = [persist.tile([128, H1], BF, tag=f"vp{j}", name=f"vp{j}")
                 for j in range(NJ)]

        with tc.tile_pool(name="ppsum", bufs=4, space=bass.MemorySpace.PSUM) as ppsum:
            # qpT[e, i] = sum_d WqT_s[d, e] * qT[d, i]  (+ s*bq)
            def qp_proj(c, psum_pool):
                src = in_ap(qT_sb, c * PCH, PCH)
                for et in range(NE):
                    esl = slice(et * 128, (et + 1) * 128)
                    pp = psum_pool.tile([128, PCH], F32, tag="pv",
                                        name=f"qpp{c}_{et}")
                    nc.tensor.matmul(pp[:], wq_sb[0][:, esl], src[0],
                                     start=True, stop=False)
                    nc.tensor.matmul(pp[:], wq_sb[1][:, esl], src[1],
                                     start=False, stop=True)
                    if (c + et) % 2 == 0:
                        nc.vector.tensor_scalar_add(qpT_sb[c][:, et, :], pp[:],
                                                    bqs_sb[et])
                    else:
                        nc.scalar.activation(qpT_sb[c][:, et, :], pp[:],
                                             AF.Identity, bias=bqs_sb[et])

            for c in range(SQ // PCH):
                qp_proj(c, ppsum)
            # kpT[e, j] = sum_d WkT[d, e] * kT[d, j]  (+ bk)
            for c in range(S // PCH):
                src = in_ap(kT_sb, c * PCH, PCH)
                for et in range(NE):
                    esl = slice(et * 128, (et + 1) * 128)
                    pp = ppsum.tile([128, PCH], F32, tag="pp")
                    nc.tensor.matmul(pp[:], wk_sb[0][:, esl], src[0],
                                     start=True, stop=False)
                    nc.tensor.matmul(pp[:], wk_sb[1][:, esl], src[1],
                                     start=False, stop=True)
                    if (c + et) % 2 == 0:
                        nc.vector.tensor_scalar_add(kpT_sb[c][:, et, :], pp[:],
                                                    bk_sb[et])
                    else:
                        nc.scalar.activation(kpT_sb[c][:, et, :], pp[:],
                                             AF.Identity, bias=bk_sb[et])

        def kp_ap(j):
            """[128, 2, 128] DoubleRow lhsT window of kpT for j-tile j."""
            c, off = divmod(j * 128, PCH)
            return kpT_sb[c][:, :, off:off + 128]

        # --- attention ---
        def attn_chunk(ic, spsum, expp, norm, obuf, opsum, fuse_vp=None):
            qp_rhs = qpT_sb[ic]  # PCH == ICH
            otiles = [opsum.tile([128, H1], F32, tag=f"ot{it}",
                                 name=f"ot{ic}_{it}") for it in range(4)]
            for j in range(NJ):
                if fuse_vp is not None:
                    fuse_vp(j)
                sp = spsum.tile([128, ICH], F32, tag="sp")
                nc.tensor.matmul(sp[:], kp_ap(j), qp_rhs[:],
                                 start=True, stop=True,
                                 perf_mode=mybir.MatmulPerfMode.DoubleRow)
                ex = expp.tile([128, ICH], BF, tag="ex")
                nc.scalar.activation(ex[:], sp[:], AF.Exp)
                for it in range(4):
                    nc.tensor.matmul(otiles[it][:],
                                     ex[:, it * 128:(it + 1) * 128],
                                     vp_sb[j][:],
                                     start=(j == 0), stop=(j == NJ - 1),
                                     skip_group_check=True)
            for it in range(4):
                rt = norm.tile([128, 1], F32, tag="rt")
                nc.vector.reciprocal(rt[:], otiles[it][:, D:H1])
                ob = obuf.tile([128, D], F32, tag="ob")
                nc.vector.tensor_scalar_mul(ob[:], otiles[it][:, 0:D], rt[:])
                r0 = (ic * 4 + it) * 128
                nc.sync.dma_start(out[r0:r0 + 128, :], ob[:])

        with tc.tile_pool(name="opsum", bufs=1, space=bass.MemorySpace.PSUM) as opsum, \
             tc.tile_pool(name="expp", bufs=4) as expp, \
             tc.tile_pool(name="norm", bufs=4) as norm, \
             tc.tile_pool(name="obuf", bufs=4) as obuf:
            # ic0: vp projection fused per-j (copies on DVE) so nothing drains
            # serially before attention starts
            with tc.tile_pool(name="spsum0", bufs=2,
                              space=bass.MemorySpace.PSUM) as spsum0, \
                 tc.tile_pool(name="vpsum", bufs=2,
                              space=bass.MemorySpace.PSUM) as vpsum:
                def vp_proj(j):
                    src2 = in_ap(vT_sb, j * 128, 128)
                    pv = vpsum.tile([128, H1], F32, tag="pv", name=f"pv{j}")
                    nc.tensor.matmul(pv[:], src2[0], wv_sb[0],
                                     start=True, stop=False, skip_group_check=True)
                    nc.tensor.matmul(pv[:], src2[1], wv_sb[1],
                                     start=False, stop=True, skip_group_check=True)
                    nc.vector.tensor_tensor(vp_sb[j][:], pv[:], biasones[:],
                                            mybir.AluOpType.add)

                # prefetch distance 2 so the vp copy never gates the AV of
                # the same iteration
                vp_proj(0)
                vp_proj(1)

                def fuse_vp(j):
                    if j + 2 < NJ:
                        vp_proj(j + 2)

                attn_chunk(0, spsum0, expp, norm, obuf, opsum, fuse_vp=fuse_vp)
            with tc.tile_pool(name="spsum1", bufs=4,
                              space=bass.MemorySpace.PSUM) as spsum1:
                for ic in range(1, NIC):
                    attn_chunk(ic, spsum1, expp, norm, obuf, opsum)

    nc.compile()
    return nc


class TileKernel:
    """Helper: TileContext + ExitStack as one context manager."""

    def __init__(self, nc, tile_mod):
        self.nc = nc
        self.tile_mod = tile_mod
        self.stack = ExitStack()

    def __enter__(self):
        tc = self.stack.enter_context(
            self.tile_mod.TileContext(self.nc, pool_alloc_mode="queue"))
        return tc, self.stack

    def __exit__(self, *exc):
        return self.stack.__exit__(*exc)


def _get_nc():
    if "nc" not in _CACHE:
        _CACHE["nc"] = _build()
    return _CACHE["nc"]


def kernel(q, k, v, Wq, bq, Wk, bk, Wv, bv):
    from concourse.bass_utils import run_bass_kernel_spmd

    q = np.asarray(q, dtype=np.float32)
    k = np.asarray(k, dtype=np.float32)
    v = np.asarray(v, dtype=np.float32)
    Wq = np.asarray(Wq, dtype=np.float32)
    bq = np.asarray(bq, dtype=np.float32)
    Wk = np.asarray(Wk, dtype=np.float32)
    bk = np.asarray(bk, dtype=np.float32)
    Wv = np.asarray(Wv, dtype=np.float32)
    bv = np.asarray(bv, dtype=np.float32)

    bf = ml_dtypes.bfloat16
    s = 1.0 / math.sqrt(D)

    WqT = (s * Wq.T).astype(bf)          # [d, e], softmax scale folded in
    WkT = Wk.T.astype(bf)                # [d, e]
    WvT = Wv.T.astype(bf)                # [d, h]
    bv1 = np.concatenate([bv, np.ones(1, np.float32)]).reshape(1, H1).astype(bf)

    blob = np.zeros((128, BLOBW), bf)
    for dt in range(NE):
        blob[:, dt * D:(dt + 1) * D] = WqT[dt * 128:(dt + 1) * 128, :]
        blob[:, 2 * D + dt * D:2 * D + (dt + 1) * D] = WkT[dt * 128:(dt + 1) * 128, :]
        blob[:, 4 * D + dt * H1:4 * D + dt * H1 + D] = WvT[dt * 128:(dt + 1) * 128, :]
    bblob = np.zeros((128, 4), np.float32)
    bqs_full = (s * bq).astype(np.float32)
    for et in range(NE):
        bblob[:, et] = bqs_full[et * 128:(et + 1) * 128]
        bblob[:, 2 + et] = bk[et * 128:(et + 1) * 128]

    shared = dict(blob=blob, bblob=bblob, bv1=bv1)
    in_maps = []
    for core in range(8):
        b, half = divmod(core, 2)
        qs = slice(half * SQ, (half + 1) * SQ)
        in_maps.append(dict(
            qT=q[b, qs, :].T.astype(bf),
            kT=k[b].T.astype(bf),
            vT=v[b].T.astype(bf),
            **shared,
        ))

    global _last_in_maps
    _last_in_maps = in_maps

    nc = _get_nc()
    res = None
    for attempt in range(3):
        try:
            res = run_bass_kernel_spmd(nc, in_maps, core_ids=list(range(8)))
            break
        except Exception:
            if attempt == 2:
                raise
            time.sleep(75)  # axon terminal occasionally wedges; it self-heals

    full = np.empty((B, S, D), np.float32)
    for core in range(8):
        b, half = divmod(core, 2)
        full[b, half * SQ:(half + 1) * SQ, :] = res.results[core]["out"]
    return full
